# revision 14
# baseline (speedup 1.0000x reference)
"""Trainium2 Bass kernel: ExponentialConcordanceLoss over all pairs.

loss = sum_{i,j: d_i < d_j, e_i = 1} exp(p_j - p_i)  /  #{such pairs}

Strategy (8 NeuronCores, SPMD): tile the 8192x8192 pair matrix as
4 j-blocks x 2 i-blocks; each core owns a 2048-j x 4096-i rectangle.
Using separability exp(p_j - p_i) = exp(p_j) * exp(-p_i):

  per core:  S_j = sum_i [d_i < d_j] * (e_i * exp(-p_i))
             R_j = sum_i [d_i < d_j] * e_i
             partials = (sum_j exp(p_j) * S_j,  sum_j R_j)

The [d_i < d_j] comparison masks ([128 i x 2048 j] per i-tile; FD=2048
amortizes the ~170-cycle DVE instruction overhead better than FD=1024)
are generated on TWO engines concurrently, interleaved in tile order so
both stream without pool-slot stalls (t%4==3 -> Scalar, else Vector):
  - Vector:  tensor_scalar is_gt (bf16 4x mode, ~740ns)   -> {0, 1}
  - Scalar:  Sign(d_j - d_i) (~2.0us)                      -> {-1, 0, +1}
(Sign shares the Exp ACT table -> single table load.  GpSimd measures
~16us/tile for this op — never route mask work there.)
The masked sums run on the Tensor engine as matmuls with [c_i, e_i]
stationaries (M=2), packed 4-wide across PE column groups
(tile_position); PSUM caps matmul output at one 512-fp32 bank, so each
tile issues 4 chunk matmuls (PE cost is ~163ns fixed + 0.417ns/col,
~2.3x effective column-group overlap -> the PE runs just behind the
mask engines).  Scalar-engine tiles use a 0.5x stationary; the constant
deficit is added back in the epilogue: L += 0.5*C_act*G,
T += 0.5*E_act*J where C_act/E_act are ce-sums over the Scalar-assigned
i-tiles (strided slice) and G = sum_j exp(p_j).

exp(p_j) is computed in column layout [128, 16] on ACT (FD=16, ~200ns
instead of ~1.9us at FD=2048); the j axis is host-permuted so a single
2D DMA gathers the column tile back into the [1, 2048] row the epilogue
needs.  G rides the same op's accum_out and a tiny ones-matmul.

DMA descriptors cost ~650ns nearly independent of size on this part
(the transfer completes asynchronously at ~140GB/s per queue), so
inputs are consolidated: one [128,2048] dj descriptor, one packed
[128,114] descriptor carrying d/p/e cols + pj cols + fold columns, one
[128,2048] w-seed canvas.  Prep work overlaps the dj transfer.

Per-core partials are summed on the host (a device AllReduce measures
~45us for 8 bytes on this fabric).

Implementation notes:
 - Every compute instruction may carry at most ONE new-semaphore sync
   wait; tiny "touch" ops absorb DMA waits per consuming engine.
 - tensor_tensor_reduce mis-executes on this runtime (and tensor_tensor
   has no accum_out); epilogue uses mul + (ACT accum || DVE reduce).
 - DMA cannot read PSUM on this runtime (src must be SBUF/DRAM).
 - PSUM partitions outside the 4 column-group windows are zero-filled by
   M=128 zero matmuls; the w canvas arrives zero-seeded from the host so
   dead rows multiply to 0.0 (one PSUM operand per TensorTensor is ok).
"""

import numpy as np
import ml_dtypes

N = 8192
NCORES = 8
P = 128
J = 2048             # j per core (4 j-blocks)
IC = 4096            # i per core (2 i-blocks)
NT = IC // P         # 32 i-tiles of 128
JC = 512             # matmul free-dim chunk (one PSUM bank)
NCHUNK = J // JC     # 4
NG = 4               # PE column groups (tile_position packing)
JT = J // P          # 16 j-columns in col-layout
MASK_BUFS = 12       # mask pool slots (reuse; > NG so PE lag can't stall)

# packed-cols layout: d | p | e | pj | fold
_OD, _OP, _OE, _OJ, _OF = 0, NT, 2 * NT, 3 * NT, 3 * NT + JT
_COLS_W = 3 * NT + JT + 2


def _is_act_tile(t):
    return t % 4 == 3


_BF16 = ml_dtypes.bfloat16

_cached = None


def _build():
    from concourse import bacc, tile, mybir

    dt = mybir.dt
    Alu = mybir.AluOpType
    Act = mybir.ActivationFunctionType

    nc = bacc.Bacc("TRN2", target_bir_lowering=False, debug=False,
                   num_devices=NCORES)

    cols_i = nc.dram_tensor("cols", [P, _COLS_W], dt.float32,
                            kind="ExternalInput").ap()
    dj_bc = nc.dram_tensor("dj_bcast", [P, J], dt.bfloat16,
                           kind="ExternalInput").ap()
    w_seed = nc.dram_tensor("w_seed", [P, J], dt.bfloat16,
                            kind="ExternalInput").ap()
    out_d = nc.dram_tensor("out", [1, 2], dt.float32, kind="ExternalOutput").ap()

    with tile.TileContext(nc) as tc:
        with (
            tc.tile_pool(name="cpool", bufs=1) as cpool,
            tc.tile_pool(name="mpool", bufs=MASK_BUFS) as mpool,
            tc.tile_pool(name="pspool", bufs=1, space="PSUM") as pspool,
        ):
            # ---- consolidated input loads
            dj_sb = cpool.tile([P, J], dt.bfloat16)
            nc.sync.dma_start(dj_sb[:], dj_bc[:])
            cols = cpool.tile([P, _COLS_W], dt.float32)
            nc.gpsimd.dma_start(cols[:], cols_i[:])
            w4 = cpool.tile([P, J], dt.bfloat16)
            nc.gpsimd.dma_start(w4[:], w_seed[:])

            dcol = cols[:, _OD:_OD + NT]
            pcol = cols[:, _OP:_OP + NT]
            ecol = cols[:, _OE:_OE + NT]
            pjc = cols[:, _OJ:_OJ + JT]
            fold = cols[:, _OF:_OF + 2]

            # ---- touches: absorb the two input-DMA waits per engine
            scratch = cpool.tile([1, 4], dt.float32)
            nc.vector.tensor_copy(scratch[0:1, 0:1], dj_sb[0:1, 0:1])
            nc.vector.tensor_copy(scratch[0:1, 1:2], cols[0:1, 0:1])
            scratch_a = cpool.tile([1, 4], dt.float32)
            nc.scalar.activation(scratch_a[0:1, 0:1], dj_sb[0:1, 0:1], Act.Copy)
            nc.scalar.activation(scratch_a[0:1, 1:2], cols[0:1, 0:1], Act.Copy)

            # ---- c_i = e_i * exp(-p_i); per-i-tile stationary [c | e] bf16
            expnp = cpool.tile([P, NT], dt.float32)
            nc.scalar.activation(expnp[:], pcol, Act.Exp, scale=-1.0)
            # exp(p_j) in column layout; per-partition sums via accum_out
            wexp_c = cpool.tile([P, JT], dt.bfloat16)
            gpart = cpool.tile([P, 1], dt.float32)
            nc.scalar.activation(wexp_c[:], pjc, Act.Exp, accum_out=gpart[:])
            ccol = cpool.tile([P, NT], dt.float32)
            nc.vector.tensor_mul(ccol[:], expnp[:], ecol)
            ce = cpool.tile([P, NT, 2], dt.bfloat16)
            nc.vector.tensor_copy(ce[:, :, 0], ccol[:])
            nc.vector.tensor_copy(ce[:, :, 1], ecol)
            # 0.5x stationary for the +-1 (ACT) tiles: exact in bf16
            ceh = cpool.tile([P, NT, 2], dt.bfloat16)
            nc.vector.tensor_scalar(ceh[:, 3:NT:4, :], ce[:, 3:NT:4, :],
                                    0.5, None, Alu.mult)
            # ACT mask bias: -d_i
            dneg = cpool.tile([P, NT], dt.float32)
            nc.vector.tensor_scalar(dneg[:], dcol, -1.0, None, Alu.mult)

            # gather exp(p_j) column tile into the w-canvas row 0 (host
            # permuted j so the streams line up), replicate to group rows.
            nc.sync.dma_start(w4[0:1, :], wexp_c[:, :])
            for g in range(1, NG):
                nc.sync.dma_start(w4[32 * g:32 * g + 1, :], w4[0:1, :])

            # ---- pairwise masks + col-tiled matmul accumulation
            ps = pspool.tile([P, NCHUNK, JC], dt.float32, name="psacc")
            zt = cpool.tile([P, JC], dt.bfloat16)
            nc.vector.memset(zt[:], 0.0)
            for c in range(NCHUNK):
                nc.tensor.matmul(ps[:, c, :], zt[:, 0:P], zt[:],
                                 start=True, stop=False, skip_group_check=True)
            for t in range(NT):
                g = t % NG
                pr = slice(32 * g, 32 * g + 2)
                mask = mpool.tile([P, J], dt.bfloat16, tag="mask", name="mask")
                if not _is_act_tile(t):
                    nc.vector.tensor_scalar(
                        mask[:], dj_sb[:], dcol[:, t:t + 1], None, Alu.is_gt)
                    stat = ce
                else:
                    nc.scalar.activation(
                        mask[:], dj_sb[:], Act.Sign, bias=dneg[:, t:t + 1])
                    stat = ceh
                for c in range(NCHUNK):
                    nc.tensor.matmul(
                        ps[pr, c, :], stat[:, t, :],
                        mask[:, c * JC:(c + 1) * JC],
                        start=False, stop=(t >= NT - NG),
                        skip_group_check=True,
                        tile_position=(0, 32 * g))

            # ---- +-1 deficit correction: corrh = 0.5*[C_act*G ; E_act*J]
            # (emitted after the mask loop so it fills engine idle gaps)
            cae = cpool.tile([P, 2], dt.float32)
            nc.vector.tensor_reduce(cae[:, 0:1], ccol[:, 3:NT:4],
                                    mybir.AxisListType.X, Alu.add)
            nc.vector.tensor_reduce(cae[:, 1:2], ecol[:, 3:NT:4],
                                    mybir.AxisListType.X, Alu.add)
            ones128 = cpool.tile([P, 1], dt.float32)
            nc.vector.memset(ones128[:], 1.0)
            ps_ce = pspool.tile([2, 1], dt.float32)
            nc.tensor.matmul(ps_ce[:], cae[:], ones128[:],
                             start=True, stop=True)
            ps_g = pspool.tile([1, 1], dt.float32)
            nc.tensor.matmul(ps_g[:], gpart[:], ones128[:],
                             start=True, stop=True)
            ce2 = cpool.tile([2, 1], dt.float32)
            nc.vector.tensor_copy(ce2[:], ps_ce[:])
            gj2 = cpool.tile([2, 1], dt.float32)
            nc.vector.memset(gj2[:], float(J))
            nc.vector.tensor_copy(gj2[0:1, 0:1], ps_g[0:1, 0:1])
            corrh = cpool.tile([2, 1], dt.float32)
            nc.vector.tensor_mul(corrh[:], ce2[:], gj2[:])
            nc.vector.tensor_scalar(corrh[:], corrh[:], 0.5, None, Alu.mult)

            # ---- epilogue: weight PSUM by w4 (direct PSUM TT), reduce
            # over j (split ACT/DVE), fold the 4 groups, correct, emit.
            prodw = cpool.tile([P, J], dt.bfloat16)
            half = J // 2
            for c in range(NCHUNK):
                nc.vector.tensor_mul(prodw[:, c * JC:(c + 1) * JC],
                                     ps[:, c, :], w4[:, c * JC:(c + 1) * JC])
            junk = cpool.tile([P, half], dt.bfloat16)
            red4a = cpool.tile([P, 1], dt.float32)
            red4b = cpool.tile([P, 1], dt.float32)
            nc.scalar.activation(junk[:], prodw[:, 0:half], Act.Copy,
                                 accum_out=red4a[:])
            nc.vector.tensor_reduce(red4b[:], prodw[:, half:J],
                                    mybir.AxisListType.X, Alu.add)
            red4 = cpool.tile([P, 1], dt.float32)
            nc.vector.tensor_add(red4[:], red4a[:], red4b[:])
            ps_f = pspool.tile([2, 1], dt.float32)
            nc.tensor.matmul(ps_f[:], fold, red4[:], start=True, stop=True)
            redf = cpool.tile([2, 1], dt.float32)
            nc.vector.tensor_add(redf[:], ps_f[:], corrh[:])
            # emit the per-core partials; host reduces across cores
            nc.sync.dma_start(out_d[0:1, 0:2], redf[0:2, 0:1])

    nc.finalize()
    return nc


def _get_program():
    global _cached
    if _cached is None:
        _cached = _build()
    return _cached


def _reduce_output(results):
    parts = np.stack([np.asarray(r["out"], dtype=np.float64).reshape(2)
                      for r in results])
    tot = parts.sum(axis=0)
    return np.float32(tot[0] / tot[1]).reshape(())


def _shard_inputs(preds, targets):
    p = np.ascontiguousarray(np.asarray(preds, dtype=np.float32).reshape(-1))
    d = np.ascontiguousarray(np.asarray(targets[:, 0], dtype=np.float32))
    e = np.ascontiguousarray(np.asarray(targets[:, 1], dtype=np.float32))

    w_seed = np.zeros((P, J), dtype=_BF16)
    for g in range(NG):
        w_seed[32 * g + 1, :] = 1.0

    in_maps = []
    for k in range(NCORES):
        jblk, iblk = k % 4, k // 4
        jsl = slice(J * jblk, J * (jblk + 1))
        isl = slice(IC * iblk, IC * (iblk + 1))
        di, pi, ei = d[isl], p[isl], e[isl]
        cols = np.zeros((P, _COLS_W), dtype=np.float32)
        cols[:, _OD:_OD + NT] = di.reshape(NT, P).T
        cols[:, _OP:_OP + NT] = pi.reshape(NT, P).T
        cols[:, _OE:_OE + NT] = ei.reshape(NT, P).T
        # j-side: permute j within the block to j' = p*JT + t so the
        # [128, JT] col-layout exp tile DMA-gathers straight into row
        # order.  d and p use the same permutation -> self-consistent.
        cols[:, _OJ:_OJ + JT] = p[jsl].reshape(JT, P).T
        for g in range(NG):
            cols[32 * g + 0, _OF + 0] = 1.0
            cols[32 * g + 1, _OF + 1] = 1.0
        dj_perm = np.ascontiguousarray(
            d[jsl].reshape(JT, P).T.reshape(-1)).astype(_BF16)
        in_maps.append({
            "cols": cols,
            "dj_bcast": np.ascontiguousarray(
                np.broadcast_to(dj_perm[None, :], (P, J))),
            "w_seed": w_seed,
        })
    return in_maps


def _run(preds, targets, trace=False):
    from concourse import bass_utils

    nc = _get_program()
    in_maps = _shard_inputs(preds, targets)
    last_err = None
    for _attempt in range(3):
        try:
            res = bass_utils.run_bass_kernel_spmd(
                nc, in_maps, list(range(NCORES)), trace=trace)
            break
        except Exception as e:  # transient NRT device wedges recover on retry
            last_err = e
    else:
        raise last_err
    out = _reduce_output(res.results)
    return out, res


def kernel(preds, targets):
    out, _ = _run(preds, targets, trace=False)
    return out


def kernel_traced(preds, targets):
    """Returns (loss, BassKernelResults) with NTFF profiling enabled."""
    return _run(preds, targets, trace=True)


# revision 15
# speedup vs baseline: 1.1358x; 1.1358x over previous
"""Trainium2 Bass kernel: ExponentialConcordanceLoss over all pairs.

loss = sum_{i,j: d_i < d_j, e_i = 1} exp(p_j - p_i)  /  #{such pairs}

Strategy (8 NeuronCores, SPMD): tile the 8192x8192 pair matrix as
4 j-blocks x 2 i-blocks; each core owns a 2048-j x 4096-i rectangle.
Using separability exp(p_j - p_i) = exp(p_j) * exp(-p_i):

  per core:  S_j = sum_i [d_i < d_j] * (e_i * exp(-p_i))
             R_j = sum_i [d_i < d_j] * e_i
             partials = (sum_j exp(p_j) * S_j,  sum_j R_j)

The [d_i < d_j] comparison masks ([128 i x 2048 j] per i-tile; FD=2048
amortizes the ~170-cycle DVE instruction overhead better than FD=1024)
are generated on TWO engines concurrently, interleaved in tile order so
both stream without pool-slot stalls (t%4==3 -> Scalar, else Vector):
  - Vector:  tensor_scalar is_gt (bf16 4x mode, ~740ns)   -> {0, 1}
  - Scalar:  Sign(d_j - d_i) (~2.0us)                      -> {-1, 0, +1}
(Sign shares the Exp ACT table -> single table load.  GpSimd measures
~16us/tile for this op — never route mask work there.)
The masked sums run on the Tensor engine as matmuls with [c_i, e_i]
stationaries (M=2), packed 4-wide across PE column groups
(tile_position); PSUM caps matmul output at one 512-fp32 bank, so each
tile issues 4 chunk matmuls (PE cost is ~163ns fixed + 0.417ns/col,
~2.3x effective column-group overlap -> the PE runs just behind the
mask engines).  Scalar-engine tiles use a 0.5x stationary; the constant
deficit is added back in the epilogue: L += 0.5*C_act*G,
T += 0.5*E_act*J where C_act/E_act are ce-sums over the Scalar-assigned
i-tiles (strided slice) and G = sum_j exp(p_j).

exp(p_j) is computed in column layout [128, 16] on ACT (FD=16, ~200ns
instead of ~1.9us at FD=2048); the j axis is host-permuted so a single
2D DMA gathers the column tile back into the [1, 2048] row the epilogue
needs.  G rides the same op's accum_out and a tiny ones-matmul.

DMA descriptors cost ~650ns nearly independent of size on this part
(the transfer completes asynchronously at ~140GB/s per queue), so
inputs are consolidated: one [128,2048] dj descriptor, one packed
[128,114] descriptor carrying d/p/e cols + pj cols + fold columns, one
[128,2048] w-seed canvas.  Prep work overlaps the dj transfer.

Per-core partials are summed on the host (a device AllReduce measures
~45us for 8 bytes on this fabric).

Implementation notes:
 - Every compute instruction may carry at most ONE new-semaphore sync
   wait; tiny "touch" ops absorb DMA waits per consuming engine.
 - tensor_tensor_reduce mis-executes on this runtime (and tensor_tensor
   has no accum_out); epilogue uses mul + (ACT accum || DVE reduce).
 - DMA cannot read PSUM on this runtime (src must be SBUF/DRAM).
 - PSUM partitions outside the 4 column-group windows are zero-filled by
   M=128 zero matmuls; the w canvas arrives zero-seeded from the host so
   dead rows multiply to 0.0 (one PSUM operand per TensorTensor is ok).
"""

import numpy as np
import ml_dtypes

N = 8192
NCORES = 8
P = 128
J = 2048             # j per core (4 j-blocks)
IC = 4096            # i per core (2 i-blocks)
NT = IC // P         # 32 i-tiles of 128
JC = 512             # matmul free-dim chunk (one PSUM bank)
NCHUNK = J // JC     # 4
NG = 4               # PE column groups (tile_position packing)
JT = J // P          # 16 j-columns in col-layout
MASK_BUFS = 6        # mask pool slots (reuse; > NG so PE lag can't stall)

# packed-cols layout: d | p | e | pj | fold
_OD, _OP, _OE, _OJ, _OF = 0, NT, 2 * NT, 3 * NT, 3 * NT + JT
_COLS_W = 3 * NT + JT + 2


def _is_act_tile(t):
    return t % 4 == 3


_BF16 = ml_dtypes.bfloat16

_cached = None


def _build():
    from concourse import bacc, tile, mybir

    dt = mybir.dt
    Alu = mybir.AluOpType
    Act = mybir.ActivationFunctionType

    nc = bacc.Bacc("TRN2", target_bir_lowering=False, debug=False,
                   num_devices=NCORES)

    cols_i = nc.dram_tensor("cols", [P, _COLS_W], dt.float32,
                            kind="ExternalInput").ap()
    dj_bc = nc.dram_tensor("dj_bcast", [P, J], dt.bfloat16,
                           kind="ExternalInput").ap()
    w_seed = nc.dram_tensor("w_seed", [P, J], dt.bfloat16,
                            kind="ExternalInput").ap()
    out_d = nc.dram_tensor("out", [1, 2], dt.float32, kind="ExternalOutput").ap()

    with tile.TileContext(nc) as tc:
        with (
            tc.tile_pool(name="cpool", bufs=1) as cpool,
            tc.tile_pool(name="mpool", bufs=MASK_BUFS) as mpool,
            tc.tile_pool(name="pspool", bufs=1, space="PSUM") as pspool,
        ):
            # ---- consolidated input loads
            dj_sb = cpool.tile([P, J], dt.bfloat16)
            nc.sync.dma_start(dj_sb[:], dj_bc[:])
            cols = cpool.tile([P, _COLS_W], dt.float32)
            nc.gpsimd.dma_start(cols[:], cols_i[:])
            w4 = cpool.tile([P, J], dt.bfloat16)
            nc.gpsimd.dma_start(w4[:], w_seed[:])

            dcol = cols[:, _OD:_OD + NT]
            pcol = cols[:, _OP:_OP + NT]
            ecol = cols[:, _OE:_OE + NT]
            pjc = cols[:, _OJ:_OJ + JT]
            fold = cols[:, _OF:_OF + 2]

            # ---- touches: absorb the two input-DMA waits per engine
            scratch = cpool.tile([1, 4], dt.float32)
            nc.vector.tensor_copy(scratch[0:1, 0:1], dj_sb[0:1, 0:1])
            nc.vector.tensor_copy(scratch[0:1, 1:2], cols[0:1, 0:1])
            scratch_a = cpool.tile([1, 4], dt.float32)
            nc.scalar.activation(scratch_a[0:1, 0:1], dj_sb[0:1, 0:1], Act.Copy)
            nc.scalar.activation(scratch_a[0:1, 1:2], cols[0:1, 0:1], Act.Copy)

            # ---- c_i = e_i * exp(-p_i); per-i-tile stationary [c | e] bf16
            expnp = cpool.tile([P, NT], dt.float32)
            nc.scalar.activation(expnp[:], pcol, Act.Exp, scale=-1.0)
            # exp(p_j) in column layout; per-partition sums via accum_out
            wexp_c = cpool.tile([P, JT], dt.bfloat16)
            gpart = cpool.tile([P, 1], dt.float32)
            nc.scalar.activation(wexp_c[:], pjc, Act.Exp, accum_out=gpart[:])
            ccol = cpool.tile([P, NT], dt.float32)
            nc.vector.tensor_mul(ccol[:], expnp[:], ecol)
            ce = cpool.tile([P, NT, 2], dt.bfloat16)
            nc.vector.tensor_copy(ce[:, :, 0], ccol[:])
            nc.vector.tensor_copy(ce[:, :, 1], ecol)
            # 0.5x stationary for the +-1 (ACT) tiles: exact in bf16
            ceh = cpool.tile([P, NT, 2], dt.bfloat16)
            nc.vector.tensor_scalar(ceh[:, 3:NT:4, :], ce[:, 3:NT:4, :],
                                    0.5, None, Alu.mult)
            # ACT mask bias: -d_i
            dneg = cpool.tile([P, NT], dt.float32)
            nc.vector.tensor_scalar(dneg[:], dcol, -1.0, None, Alu.mult)

            # gather exp(p_j) column tile into the w-canvas row 0 (host
            # permuted j so the streams line up), replicate to group rows.
            nc.sync.dma_start(w4[0:1, :], wexp_c[:, :])
            for g in range(1, NG):
                nc.sync.dma_start(w4[32 * g:32 * g + 1, :], w4[0:1, :])

            # ---- pairwise masks + col-tiled matmul accumulation
            ps = pspool.tile([P, NCHUNK, JC], dt.float32, name="psacc")
            zt = cpool.tile([P, JC], dt.bfloat16)
            nc.vector.memset(zt[:], 0.0)
            for c in range(NCHUNK):
                nc.tensor.matmul(ps[:, c, :], zt[:, 0:P], zt[:],
                                 start=True, stop=False, skip_group_check=True)
            for t in range(NT):
                g = t % NG
                pr = slice(32 * g, 32 * g + 2)
                mask = mpool.tile([P, J], dt.bfloat16, tag="mask", name="mask")
                if not _is_act_tile(t):
                    nc.vector.tensor_scalar(
                        mask[:], dj_sb[:], dcol[:, t:t + 1], None, Alu.is_gt)
                    stat = ce
                else:
                    nc.scalar.activation(
                        mask[:], dj_sb[:], Act.Sign, bias=dneg[:, t:t + 1])
                    stat = ceh
                for c in range(NCHUNK):
                    nc.tensor.matmul(
                        ps[pr, c, :], stat[:, t, :],
                        mask[:, c * JC:(c + 1) * JC],
                        start=False, stop=(t >= NT - NG),
                        skip_group_check=True,
                        tile_position=(0, 32 * g))

            # ---- +-1 deficit correction: corrh = 0.5*[C_act*G ; E_act*J]
            # (emitted after the mask loop so it fills engine idle gaps)
            cae = cpool.tile([P, 2], dt.float32)
            nc.vector.tensor_reduce(cae[:, 0:1], ccol[:, 3:NT:4],
                                    mybir.AxisListType.X, Alu.add)
            nc.vector.tensor_reduce(cae[:, 1:2], ecol[:, 3:NT:4],
                                    mybir.AxisListType.X, Alu.add)
            ones128 = cpool.tile([P, 1], dt.float32)
            nc.vector.memset(ones128[:], 1.0)
            ps_ce = pspool.tile([2, 1], dt.float32)
            nc.tensor.matmul(ps_ce[:], cae[:], ones128[:],
                             start=True, stop=True)
            ps_g = pspool.tile([1, 1], dt.float32)
            nc.tensor.matmul(ps_g[:], gpart[:], ones128[:],
                             start=True, stop=True)
            ce2 = cpool.tile([2, 1], dt.float32)
            nc.vector.tensor_copy(ce2[:], ps_ce[:])
            gj2 = cpool.tile([2, 1], dt.float32)
            nc.vector.memset(gj2[:], float(J))
            nc.vector.tensor_copy(gj2[0:1, 0:1], ps_g[0:1, 0:1])
            corrh = cpool.tile([2, 1], dt.float32)
            nc.vector.tensor_mul(corrh[:], ce2[:], gj2[:])
            nc.vector.tensor_scalar(corrh[:], corrh[:], 0.5, None, Alu.mult)

            # ---- epilogue: weight PSUM by w4 (direct PSUM TT), reduce
            # over j (split ACT/DVE), fold the 4 groups, correct, emit.
            prodw = cpool.tile([P, J], dt.bfloat16)
            half = J // 2
            for c in range(NCHUNK):
                nc.vector.tensor_mul(prodw[:, c * JC:(c + 1) * JC],
                                     ps[:, c, :], w4[:, c * JC:(c + 1) * JC])
            junk = cpool.tile([P, half], dt.bfloat16)
            junk2 = cpool.tile([P, half], dt.bfloat16)
            red4a = cpool.tile([P, 1], dt.float32)
            red4b = cpool.tile([P, 1], dt.float32)
            nc.scalar.activation(junk[:], prodw[:, 0:half], Act.Copy,
                                 accum_out=red4a[:])
            nc.scalar.activation(junk2[:], prodw[:, half:J], Act.Copy,
                                 accum_out=red4b[:])
            red4 = cpool.tile([P, 1], dt.float32)
            nc.vector.tensor_add(red4[:], red4a[:], red4b[:])
            ps_f = pspool.tile([2, 1], dt.float32)
            nc.tensor.matmul(ps_f[:], fold, red4[:], start=True, stop=True)
            redf = cpool.tile([2, 1], dt.float32)
            nc.vector.tensor_add(redf[:], ps_f[:], corrh[:])
            # emit the per-core partials; host reduces across cores
            nc.sync.dma_start(out_d[0:1, 0:2], redf[0:2, 0:1])

    nc.finalize()
    return nc


def _get_program():
    global _cached
    if _cached is None:
        _cached = _build()
    return _cached


def _reduce_output(results):
    parts = np.stack([np.asarray(r["out"], dtype=np.float64).reshape(2)
                      for r in results])
    tot = parts.sum(axis=0)
    return np.float32(tot[0] / tot[1]).reshape(())


def _shard_inputs(preds, targets):
    p = np.ascontiguousarray(np.asarray(preds, dtype=np.float32).reshape(-1))
    d = np.ascontiguousarray(np.asarray(targets[:, 0], dtype=np.float32))
    e = np.ascontiguousarray(np.asarray(targets[:, 1], dtype=np.float32))

    w_seed = np.zeros((P, J), dtype=_BF16)
    for g in range(NG):
        w_seed[32 * g + 1, :] = 1.0

    in_maps = []
    for k in range(NCORES):
        jblk, iblk = k % 4, k // 4
        jsl = slice(J * jblk, J * (jblk + 1))
        isl = slice(IC * iblk, IC * (iblk + 1))
        di, pi, ei = d[isl], p[isl], e[isl]
        cols = np.zeros((P, _COLS_W), dtype=np.float32)
        cols[:, _OD:_OD + NT] = di.reshape(NT, P).T
        cols[:, _OP:_OP + NT] = pi.reshape(NT, P).T
        cols[:, _OE:_OE + NT] = ei.reshape(NT, P).T
        # j-side: permute j within the block to j' = p*JT + t so the
        # [128, JT] col-layout exp tile DMA-gathers straight into row
        # order.  d and p use the same permutation -> self-consistent.
        cols[:, _OJ:_OJ + JT] = p[jsl].reshape(JT, P).T
        for g in range(NG):
            cols[32 * g + 0, _OF + 0] = 1.0
            cols[32 * g + 1, _OF + 1] = 1.0
        dj_perm = np.ascontiguousarray(
            d[jsl].reshape(JT, P).T.reshape(-1)).astype(_BF16)
        in_maps.append({
            "cols": cols,
            "dj_bcast": np.ascontiguousarray(
                np.broadcast_to(dj_perm[None, :], (P, J))),
            "w_seed": w_seed,
        })
    return in_maps


def _run(preds, targets, trace=False):
    from concourse import bass_utils

    nc = _get_program()
    in_maps = _shard_inputs(preds, targets)
    last_err = None
    for _attempt in range(3):
        try:
            res = bass_utils.run_bass_kernel_spmd(
                nc, in_maps, list(range(NCORES)), trace=trace)
            break
        except Exception as e:  # transient NRT device wedges recover on retry
            last_err = e
    else:
        raise last_err
    out = _reduce_output(res.results)
    return out, res


def kernel(preds, targets):
    out, _ = _run(preds, targets, trace=False)
    return out


def kernel_traced(preds, targets):
    """Returns (loss, BassKernelResults) with NTFF profiling enabled."""
    return _run(preds, targets, trace=True)


# revision 16
# speedup vs baseline: 1.1829x; 1.0414x over previous
"""Trainium2 Bass kernel: ExponentialConcordanceLoss over all pairs.

loss = sum_{i,j: d_i < d_j, e_i = 1} exp(p_j - p_i)  /  #{such pairs}

Strategy (8 NeuronCores, SPMD): tile the 8192x8192 pair matrix as
4 j-blocks x 2 i-blocks; each core owns a 2048-j x 4096-i rectangle.
Using separability exp(p_j - p_i) = exp(p_j) * exp(-p_i):

  per core:  S_j = sum_i [d_i < d_j] * (e_i * exp(-p_i))
             R_j = sum_i [d_i < d_j] * e_i
             partials = (sum_j exp(p_j) * S_j,  sum_j R_j)

The [d_i < d_j] comparison masks ([128 i x 2048 j] per i-tile; FD=2048
amortizes the ~170-cycle DVE instruction overhead better than FD=1024)
are generated on TWO engines concurrently, interleaved in tile order so
both stream without pool-slot stalls (t%4==3 -> Scalar, else Vector):
  - Vector:  tensor_scalar is_gt (bf16 4x mode, ~740ns)   -> {0, 1}
  - Scalar:  Sign(d_j - d_i) (~2.0us)                      -> {-1, 0, +1}
(Sign shares the Exp ACT table -> single table load.  GpSimd measures
~16us/tile for this op — never route mask work there.)
The masked sums run on the Tensor engine as matmuls with [c_i, e_i]
stationaries (M=2), packed 4-wide across PE column groups
(tile_position); PSUM caps matmul output at one 512-fp32 bank, so each
tile issues 4 chunk matmuls (PE cost is ~163ns fixed + 0.417ns/col,
~2.3x effective column-group overlap -> the PE runs just behind the
mask engines).  Scalar-engine tiles use a 0.5x stationary; the constant
deficit is added back in the epilogue: L += 0.5*C_act*G,
T += 0.5*E_act*J where C_act/E_act are ce-sums over the Scalar-assigned
i-tiles (strided slice) and G = sum_j exp(p_j).

exp(p_j) is computed in column layout [128, 16] on ACT (FD=16, ~200ns
instead of ~1.9us at FD=2048); the j axis is host-permuted so a single
2D DMA gathers the column tile back into the [1, 2048] row the epilogue
needs.  G rides the same op's accum_out and a tiny ones-matmul.

DMA descriptors cost ~650ns nearly independent of size on this part
(the transfer completes asynchronously at ~140GB/s per queue), so
inputs are consolidated: one [128,2048] dj descriptor, one packed
[128,114] descriptor carrying d/p/e cols + pj cols + fold columns, one
[128,2048] w-seed canvas.  Prep work overlaps the dj transfer.

Per-core partials are summed on the host (a device AllReduce measures
~45us for 8 bytes on this fabric).

Implementation notes:
 - Every compute instruction may carry at most ONE new-semaphore sync
   wait; tiny "touch" ops absorb DMA waits per consuming engine.
 - tensor_tensor_reduce mis-executes on this runtime (and tensor_tensor
   has no accum_out); epilogue uses mul + (ACT accum || DVE reduce).
 - DMA cannot read PSUM on this runtime (src must be SBUF/DRAM).
 - PSUM partitions outside the 4 column-group windows are zero-filled by
   M=128 zero matmuls; the w canvas arrives zero-seeded from the host so
   dead rows multiply to 0.0 (one PSUM operand per TensorTensor is ok).
"""

import numpy as np
import ml_dtypes

N = 8192
NCORES = 8
P = 128
J = 2048             # j per core (4 j-blocks)
IC = 4096            # i per core (2 i-blocks)
NT = IC // P         # 32 i-tiles of 128
JC = 512             # matmul free-dim chunk (one PSUM bank)
NCHUNK = J // JC     # 4
NG = 4               # PE column groups (tile_position packing)
JT = J // P          # 16 j-columns in col-layout
MASK_BUFS = 12       # mask pool slots (reuse; > NG so PE lag can't stall)

# packed-cols layout: d | p | e | pj | fold
_OD, _OP, _OE, _OJ, _OF = 0, NT, 2 * NT, 3 * NT, 3 * NT + JT
_COLS_W = 3 * NT + JT + 2


def _is_act_tile(t):
    return t % 4 == 3


_BF16 = ml_dtypes.bfloat16

_cached = None


def _build():
    from concourse import bacc, tile, mybir

    dt = mybir.dt
    Alu = mybir.AluOpType
    Act = mybir.ActivationFunctionType

    nc = bacc.Bacc("TRN2", target_bir_lowering=False, debug=False,
                   num_devices=NCORES)

    cols_i = nc.dram_tensor("cols", [P, _COLS_W], dt.float32,
                            kind="ExternalInput").ap()
    dj_bc = nc.dram_tensor("dj_bcast", [P, J], dt.bfloat16,
                           kind="ExternalInput").ap()
    w_seed = nc.dram_tensor("w_seed", [P, J], dt.bfloat16,
                            kind="ExternalInput").ap()
    out_d = nc.dram_tensor("out", [1, 2], dt.float32, kind="ExternalOutput").ap()

    with tile.TileContext(nc) as tc:
        with (
            tc.tile_pool(name="cpool", bufs=1) as cpool,
            tc.tile_pool(name="mpool", bufs=MASK_BUFS) as mpool,
            tc.tile_pool(name="pspool", bufs=1, space="PSUM") as pspool,
        ):
            # ---- consolidated input loads
            dj_sb = cpool.tile([P, J], dt.bfloat16)
            nc.sync.dma_start(dj_sb[:], dj_bc[:])
            cols = cpool.tile([P, _COLS_W], dt.float32)
            nc.gpsimd.dma_start(cols[:], cols_i[:])
            w4 = cpool.tile([P, J], dt.bfloat16)
            nc.gpsimd.dma_start(w4[:], w_seed[:])

            dcol = cols[:, _OD:_OD + NT]
            pcol = cols[:, _OP:_OP + NT]
            ecol = cols[:, _OE:_OE + NT]
            pjc = cols[:, _OJ:_OJ + JT]
            fold = cols[:, _OF:_OF + 2]

            # ---- touches: absorb the two input-DMA waits per engine
            scratch = cpool.tile([1, 4], dt.float32)
            nc.vector.tensor_copy(scratch[0:1, 0:1], dj_sb[0:1, 0:1])
            nc.vector.tensor_copy(scratch[0:1, 1:2], cols[0:1, 0:1])
            scratch_a = cpool.tile([1, 4], dt.float32)
            nc.scalar.activation(scratch_a[0:1, 0:1], dj_sb[0:1, 0:1], Act.Copy)
            nc.scalar.activation(scratch_a[0:1, 1:2], cols[0:1, 0:1], Act.Copy)

            # ---- c_i = e_i * exp(-p_i); per-i-tile stationary [c | e] bf16
            expnp = cpool.tile([P, NT], dt.float32)
            nc.scalar.activation(expnp[:], pcol, Act.Exp, scale=-1.0)
            # exp(p_j) in column layout; per-partition sums via accum_out
            wexp_c = cpool.tile([P, JT], dt.bfloat16)
            gpart = cpool.tile([P, 1], dt.float32)
            nc.scalar.activation(wexp_c[:], pjc, Act.Exp, accum_out=gpart[:])
            ccol = cpool.tile([P, NT], dt.float32)
            nc.vector.tensor_mul(ccol[:], expnp[:], ecol)
            ce = cpool.tile([P, NT, 2], dt.bfloat16)
            nc.vector.tensor_copy(ce[:, :, 0], ccol[:])
            nc.vector.tensor_copy(ce[:, :, 1], ecol)
            # 0.5x stationary for the +-1 (ACT) tiles: exact in bf16
            ceh = cpool.tile([P, NT, 2], dt.bfloat16)
            nc.vector.tensor_scalar(ceh[:, 3:NT:4, :], ce[:, 3:NT:4, :],
                                    0.5, None, Alu.mult)
            # ACT mask bias: -d_i
            dneg = cpool.tile([P, NT], dt.float32)
            nc.vector.tensor_scalar(dneg[:], dcol, -1.0, None, Alu.mult)

            # gather exp(p_j) column tile into the w-canvas row 0 (host
            # permuted j so the streams line up), replicate to group rows.
            nc.sync.dma_start(w4[0:1, :], wexp_c[:, :])
            for g in range(1, NG):
                nc.sync.dma_start(w4[32 * g:32 * g + 1, :], w4[0:1, :])

            # ---- pairwise masks + col-tiled matmul accumulation
            ps = pspool.tile([P, NCHUNK, JC], dt.float32, name="psacc")
            zt = cpool.tile([P, JC], dt.bfloat16)
            nc.vector.memset(zt[:], 0.0)
            for c in range(NCHUNK):
                nc.tensor.matmul(ps[:, c, :], zt[:, 0:P], zt[:],
                                 start=True, stop=False, skip_group_check=True)
            for t in range(NT):
                g = t % NG
                pr = slice(32 * g, 32 * g + 2)
                mask = mpool.tile([P, J], dt.bfloat16, tag="mask", name="mask")
                if not _is_act_tile(t):
                    nc.vector.tensor_scalar(
                        mask[:], dj_sb[:], dcol[:, t:t + 1], None, Alu.is_gt)
                    stat = ce
                else:
                    nc.scalar.activation(
                        mask[:], dj_sb[:], Act.Sign, bias=dneg[:, t:t + 1])
                    stat = ceh
                for c in range(NCHUNK):
                    nc.tensor.matmul(
                        ps[pr, c, :], stat[:, t, :],
                        mask[:, c * JC:(c + 1) * JC],
                        start=False, stop=(t >= NT - NG),
                        skip_group_check=True,
                        tile_position=(0, 32 * g))

            # ---- +-1 deficit correction: corrh = 0.5*[C_act*G ; E_act*J]
            # (emitted after the mask loop so it fills engine idle gaps)
            cae = cpool.tile([P, 2], dt.float32)
            nc.vector.tensor_reduce(cae[:, 0:1], ccol[:, 3:NT:4],
                                    mybir.AxisListType.X, Alu.add)
            nc.vector.tensor_reduce(cae[:, 1:2], ecol[:, 3:NT:4],
                                    mybir.AxisListType.X, Alu.add)
            ones128 = cpool.tile([P, 1], dt.float32)
            nc.vector.memset(ones128[:], 1.0)
            ps_ce = pspool.tile([2, 1], dt.float32)
            nc.tensor.matmul(ps_ce[:], cae[:], ones128[:],
                             start=True, stop=True)
            ps_g = pspool.tile([1, 1], dt.float32)
            nc.tensor.matmul(ps_g[:], gpart[:], ones128[:],
                             start=True, stop=True)
            ce2 = cpool.tile([2, 1], dt.float32)
            nc.vector.tensor_copy(ce2[:], ps_ce[:])
            gj2 = cpool.tile([2, 1], dt.float32)
            nc.vector.memset(gj2[:], float(J))
            nc.vector.tensor_copy(gj2[0:1, 0:1], ps_g[0:1, 0:1])
            corrh = cpool.tile([2, 1], dt.float32)
            nc.vector.tensor_mul(corrh[:], ce2[:], gj2[:])
            nc.vector.tensor_scalar(corrh[:], corrh[:], 0.5, None, Alu.mult)

            # ---- epilogue: weight PSUM by w4 (direct PSUM TT), reduce
            # over j (split ACT/DVE), fold the 4 groups, correct, emit.
            prodw = cpool.tile([P, J], dt.bfloat16)
            half = J // 2
            for c in range(NCHUNK):
                nc.vector.tensor_mul(prodw[:, c * JC:(c + 1) * JC],
                                     ps[:, c, :], w4[:, c * JC:(c + 1) * JC])
            junk = cpool.tile([P, half], dt.bfloat16)
            red4a = cpool.tile([P, 1], dt.float32)
            red4b = cpool.tile([P, 1], dt.float32)
            nc.scalar.activation(junk[:], prodw[:, 0:half], Act.Copy,
                                 accum_out=red4a[:])
            nc.vector.tensor_reduce(red4b[:], prodw[:, half:J],
                                    mybir.AxisListType.X, Alu.add)
            red4 = cpool.tile([P, 1], dt.float32)
            nc.vector.tensor_add(red4[:], red4a[:], red4b[:])
            ps_f = pspool.tile([2, 1], dt.float32)
            nc.tensor.matmul(ps_f[:], fold, red4[:], start=True, stop=True)
            redf = cpool.tile([2, 1], dt.float32)
            nc.vector.tensor_add(redf[:], ps_f[:], corrh[:])
            # emit the per-core partials; host reduces across cores
            nc.sync.dma_start(out_d[0:1, 0:2], redf[0:2, 0:1])

    nc.finalize()
    return nc


def _get_program():
    global _cached
    if _cached is None:
        _cached = _build()
    return _cached


def _reduce_output(results):
    parts = np.stack([np.asarray(r["out"], dtype=np.float64).reshape(2)
                      for r in results])
    tot = parts.sum(axis=0)
    return np.float32(tot[0] / tot[1]).reshape(())


def _shard_inputs(preds, targets):
    p = np.ascontiguousarray(np.asarray(preds, dtype=np.float32).reshape(-1))
    d = np.ascontiguousarray(np.asarray(targets[:, 0], dtype=np.float32))
    e = np.ascontiguousarray(np.asarray(targets[:, 1], dtype=np.float32))

    w_seed = np.zeros((P, J), dtype=_BF16)
    for g in range(NG):
        w_seed[32 * g + 1, :] = 1.0

    in_maps = []
    for k in range(NCORES):
        jblk, iblk = k % 4, k // 4
        jsl = slice(J * jblk, J * (jblk + 1))
        isl = slice(IC * iblk, IC * (iblk + 1))
        di, pi, ei = d[isl], p[isl], e[isl]
        cols = np.zeros((P, _COLS_W), dtype=np.float32)
        cols[:, _OD:_OD + NT] = di.reshape(NT, P).T
        cols[:, _OP:_OP + NT] = pi.reshape(NT, P).T
        cols[:, _OE:_OE + NT] = ei.reshape(NT, P).T
        # j-side: permute j within the block to j' = p*JT + t so the
        # [128, JT] col-layout exp tile DMA-gathers straight into row
        # order.  d and p use the same permutation -> self-consistent.
        cols[:, _OJ:_OJ + JT] = p[jsl].reshape(JT, P).T
        for g in range(NG):
            cols[32 * g + 0, _OF + 0] = 1.0
            cols[32 * g + 1, _OF + 1] = 1.0
        dj_perm = np.ascontiguousarray(
            d[jsl].reshape(JT, P).T.reshape(-1)).astype(_BF16)
        in_maps.append({
            "cols": cols,
            "dj_bcast": np.ascontiguousarray(
                np.broadcast_to(dj_perm[None, :], (P, J))),
            "w_seed": w_seed,
        })
    return in_maps


def _run(preds, targets, trace=False):
    from concourse import bass_utils

    nc = _get_program()
    in_maps = _shard_inputs(preds, targets)
    last_err = None
    for _attempt in range(3):
        try:
            res = bass_utils.run_bass_kernel_spmd(
                nc, in_maps, list(range(NCORES)), trace=trace)
            break
        except Exception as e:  # transient NRT device wedges recover on retry
            last_err = e
    else:
        raise last_err
    out = _reduce_output(res.results)
    return out, res


def kernel(preds, targets):
    out, _ = _run(preds, targets, trace=False)
    return out


def kernel_traced(preds, targets):
    """Returns (loss, BassKernelResults) with NTFF profiling enabled."""
    return _run(preds, targets, trace=True)
